# revision 20
# baseline (speedup 1.0000x reference)
"""Trainium2 Bass kernel for the DVSA loss function.

Contract: kernel(**inputs) takes the FULL unsharded inputs
(vis_feats (51200,512) f32, word_feats (192,512) f32, entities_length (16,)
int, Nb scalar) and returns the full outputs (D_ind, D_sim, margin_loss),
matching reference semantics.

Strategy: shard vis_feats rows 8 ways (data-parallel over actions);
word_feats is masked (padding entity slots zeroed from entities_length) and
replicated. Each core computes S^T = Wm @ V^T via the tensor engine with the
box-group max/argmax fused on-chip; the tiny loss tail runs on host from the
(Na*Ns, Na*Ne) max/argmax arrays.
"""

import numpy as np

EPS = 1e-5
DELTA = 0.2
VIS_LAM = 1.0

# Geometry (hardcoded per problem spec).
NA, NS, NB, NE, D = 16, 32, 100, 12, 512
N_CORES = 8
ROWS = NA * NS * NB            # 51200
R_CORE = ROWS // N_CORES       # 6400 rows/core
F_CORE = R_CORE // NB          # 64 frames/core
COLS = NA * NE                 # 192
KD = D // 128                  # 4 contraction chunks
CF = 5                         # frames per PSUM chunk (500 rows <= 512 fp32 bank)
H0 = 128                       # first column-half partitions
H1 = COLS - H0                 # second column-half partitions (64)

# Matmul precision mode: 'f32r' (1 cyc/row), 'f16x2' (hi/lo, 3 cyc/row),
# 'f32' (4 cyc/row, exact).
MODE = "f32r"

_NC_CACHE = {}


def _build_nc(mode=MODE, r_core=R_CORE, nb=NB, cols=COLS, d=D, cf=CF,
              ablate=None, reps=1):
    import concourse.bacc as bacc
    import concourse.bass as bass
    import concourse.tile as tile
    from concourse import mybir

    f32 = mybir.dt.float32
    kd = d // 128
    n_frames = r_core // nb
    ng = (cols + 127) // 128          # column groups, zero-padded to 128 each
    cpad = ng * 128

    nc = bacc.Bacc("TRN2", target_bir_lowering=False, debug=False)

    if mode == "f16x2":
        mdt = mybir.dt.float16
        v_drams = [
            nc.dram_tensor("vth", [d, r_core], mdt, kind="ExternalInput"),
            nc.dram_tensor("vtl", [d, r_core], mdt, kind="ExternalInput"),
        ]
        w_drams = [
            nc.dram_tensor("wth", [kd, 128, cpad], mdt, kind="ExternalInput"),
            nc.dram_tensor("wtl", [kd, 128, cpad], mdt, kind="ExternalInput"),
        ]
    else:
        mdt = f32 if mode == "f32" else mybir.dt.float32r
        v_drams = [nc.dram_tensor("vt", [d, r_core], mdt, kind="ExternalInput")]
        w_drams = [nc.dram_tensor("wt", [kd, 128, cpad], mdt, kind="ExternalInput")]

    u32 = mybir.dt.uint32
    iota_d = nc.dram_tensor("iota", [128, cf * nb + 1], u32, kind="ExternalInput")
    sim_d = nc.dram_tensor("sim0", [128, ng, n_frames], f32, kind="ExternalOutput")

    with tile.TileContext(nc) as tc:
        with (
            tc.tile_pool(name="const", bufs=1) as const_pool,
            tc.tile_pool(name="vin", bufs=6) as vin_pool,
            tc.tile_pool(name="work", bufs=3) as work_pool,
            tc.tile_pool(name="psp", bufs=4, space=bass.MemorySpace.PSUM) as ps_pool,
        ):
            w_sbs = []
            for wi, wd in enumerate(w_drams):
                w_sb = const_pool.tile([128, kd, cpad], mdt, tag=f"w{wi}", name=f"w{wi}")
                for k in range(kd):
                    nc.sync.dma_start(out=w_sb[:, k, :], in_=wd[k])
                w_sbs.append(w_sb)

            iota_sb = const_pool.tile([128, cf * nb + 1], u32)
            nc.sync.dma_start(out=iota_sb[:], in_=iota_d[:])

            osim = const_pool.tile([128, ng, n_frames], f32, name="osim")
            if ablate:
                nc.vector.memset(osim[:], 0.0)

            n_chunks = (n_frames + cf - 1) // cf
            for rep in range(reps):
              for ci0 in range(n_chunks):
                ci = f"{rep}_{ci0}" if reps > 1 else ci0
                f0 = ci0 * cf
                nf = min(cf, n_frames - f0)
                r0 = f0 * nb
                rn = nf * nb

                v_sbs = []
                for vi, vd in enumerate(v_drams):
                    v_sb = vin_pool.tile([128, kd, cf * nb], mdt, tag=f"v{vi}", name=f"v{vi}_{ci}")
                    for k in range(kd):
                        nc.sync.dma_start(out=v_sb[:, k, :rn],
                                          in_=vd[k * 128:(k + 1) * 128, r0:r0 + rn])
                    v_sbs.append(v_sb)

                if mode == "f16x2":
                    # S = Vh@Wh + Vl@Wh + Vh@Wl (drop Vl@Wl, ~2^-22 relative)
                    mm_pairs = [(w_sbs[0], v_sbs[0]), (w_sbs[0], v_sbs[1]),
                                (w_sbs[1], v_sbs[0])]
                else:
                    mm_pairs = [(w_sbs[0], v_sbs[0])]

                if ablate == "dma":
                    continue
                # one PSUM tile spanning ng banks; col-group g -> bank g
                ps = ps_pool.tile([128, ng, 512], f32, tag="ps", name=f"ps_{ci}")
                n_mm = kd * len(mm_pairs)
                for g in range(ng):
                    i_mm = 0
                    for k in range(kd):
                        for (w_sb, v_sb) in mm_pairs:
                            nc.tensor.matmul(
                                ps[:, g, :rn],
                                w_sb[:, k, g * 128:(g + 1) * 128],
                                v_sb[:, k, :rn],
                                start=(i_mm == 0),
                                stop=(i_mm == n_mm - 1),
                            )
                            i_mm += 1

                if ablate == "pe":
                    continue
                # Bit-embed the descending box index into the 7 low mantissa
                # bits: emb = (S & ~0x7F) | (nb-1-box). fp32 max over emb
                # then yields max (top bits, ~2^-17 truncation) AND
                # first-occurrence argmax (low bits) in one reduce.
                emb = work_pool.tile([128, ng, cf * nb], u32, tag="emb", name=f"emb_{ci}")
                embv = emb[:, :, :rn]
                psv = ps[:, :, :rn]
                iota_bcast = bass.AP(
                    tensor=iota_sb.tensor, offset=iota_sb.offset,
                    ap=[[iota_sb.ap[0][0], 128], [0, ng], [1, rn]],
                )
                nc.vector.scalar_tensor_tensor(
                    out=embv, in0=psv.bitcast(u32),
                    scalar=iota_sb[:, cf * nb:cf * nb + 1],
                    in1=iota_bcast,
                    op0=mybir.AluOpType.bitwise_and,
                    op1=mybir.AluOpType.bitwise_or,
                )
                nc.vector.tensor_reduce(
                    osim[:, :, f0:f0 + nf],
                    emb[:, :, :rn].rearrange("p t (f b) -> p t f b", b=nb).bitcast(f32),
                    axis=mybir.AxisListType.X, op=mybir.AluOpType.max,
                )

            nc.sync.dma_start(out=sim_d[:], in_=osim[:])

    nc.compile()
    return nc


def _get_nc(mode=MODE):
    if mode not in _NC_CACHE:
        _NC_CACHE[mode] = _build_nc(mode)
    return _NC_CACHE[mode]


def _split_f16(x):
    hi = x.astype(np.float16)
    lo = (x - hi.astype(np.float32)).astype(np.float16)
    return hi, lo


def _prep_in_maps(vis, wm, mode=MODE):
    """vis: (ROWS, D) f32; wm: masked word_feats (COLS, D) f32."""
    iota_desc = np.empty((128, CF * NB + 1), dtype=np.uint32)
    iota_desc[:, :CF * NB] = np.tile(
        np.arange(NB - 1, -1, -1, dtype=np.uint32), CF)[None, :]
    iota_desc[:, CF * NB] = np.uint32(0xFFFFFF80)
    ng = (COLS + 127) // 128
    wmp = np.zeros((ng * 128, D), dtype=np.float32)
    wmp[:COLS] = wm
    wmt = np.ascontiguousarray(wmp.T).reshape(KD, 128, ng * 128)
    in_maps = []
    for c in range(N_CORES):
        vt = np.ascontiguousarray(vis[c * R_CORE:(c + 1) * R_CORE].T)
        m = {"iota": iota_desc}
        if mode == "f16x2":
            vh, vl = _split_f16(vt)
            wh, wl = _split_f16(wmt)
            m.update({"vth": vh, "vtl": vl, "wth": wh, "wtl": wl})
        else:
            m.update({"vt": vt, "wt": wmt})
        in_maps.append(m)
    return in_maps


def _run_device(vis, wm, mode=MODE, trace=False):
    from concourse.bass_utils import run_bass_kernel_spmd

    nc = _get_nc(mode)
    in_maps = _prep_in_maps(vis, wm, mode)
    res = run_bass_kernel_spmd(nc, in_maps, list(range(N_CORES)), trace=trace)
    embs = []
    for c in range(N_CORES):
        st = res.results[c]["sim0"]                           # [128, ng, 64]
        full = np.concatenate([st[:, g, :] for g in range(st.shape[1])], axis=0)
        embs.append(full[:COLS].T)                            # [64, 192]
    raw = np.ascontiguousarray(np.concatenate(embs, axis=0)).view(np.uint32)
    d_sim = (raw & np.uint32(0xFFFFFF80)).view(np.float32)    # (512, 192)
    d_ind = (NB - 1) - (raw & np.uint32(0x7F)).astype(np.int64)
    return d_sim, d_ind, res


def _host_tail(vis, ent, col_mask, d_sim, d_ind):
    """Mirror of the reference loss tail, in float64, from D_sim/D_ind."""
    na, ns, ne = NA, NS, NE
    ar = np.arange(na)

    div_vec = np.where(ent == 0, 1, ent).astype(np.float64)

    # --- vis loss ---
    ds4 = d_sim.reshape(na, ns, na, ne)
    di4 = d_ind.reshape(na, ns, na, ne)
    sim_scr = ds4[ar, :, ar, :].astype(np.float64)            # (Na, Ns, Ne)
    maxind = di4[ar, :, ar, :]                                # (Na, Ns, Ne)
    smin = sim_scr.min(axis=1, keepdims=True)
    smax = sim_scr.max(axis=1, keepdims=True)
    sim_n = (sim_scr - smin) / (smax - smin + EPS)

    vcls = vis[maxind.reshape(-1)].reshape(na, ns, ne, D).astype(np.float64)
    vcls = vcls / (np.linalg.norm(vcls, axis=3, keepdims=True) + EPS)
    vcls = vcls * sim_n[..., None]
    v1 = vcls.transpose(0, 2, 1, 3)                           # (Na, Ne, Ns, D)
    vis_mat = 1.0 - np.einsum("aesd,aetd->aest", v1, v1)
    eye = np.eye(ns, dtype=bool)
    mvis = col_mask[:, :, None, None] | eye[None, None, :, :]
    vis_mat = np.where(mvis, 0.0, vis_mat)
    dem = np.count_nonzero(vis_mat)
    vis_loss = vis_mat.sum() / dem

    # --- margin loss ---
    s = d_sim.reshape(na, ns, na * ne).astype(np.float64)
    smin2 = s.min(axis=1, keepdims=True)
    smax2 = s.max(axis=1, keepdims=True)
    s = s * ((s - smin2) / (smax2 - smin2 + EPS))
    sf = s.reshape(na, ns, na, ne).sum(-1) / div_vec          # (Na, Ns, Na)
    sf_diag = sf[ar, :, ar]                                   # (Na, Ns)
    term1 = np.maximum(sf - sf_diag.T[None, :, :] + DELTA, 0.0).mean(0).T
    term2 = np.maximum(sf - sf_diag[:, :, None] + DELTA, 0.0).mean(2)
    frame_score = term1 + term2
    margin_loss = (frame_score.mean() + VIS_LAM * vis_loss) * 10.0
    return np.float32(margin_loss)


def kernel(vis_feats, word_feats, entities_length, Nb):
    vis = np.ascontiguousarray(np.asarray(vis_feats, dtype=np.float32))
    wf = np.asarray(word_feats, dtype=np.float32)
    ent = np.asarray(entities_length)
    assert int(Nb) == NB and vis.shape == (ROWS, D) and wf.shape == (COLS, D)

    col_mask = np.arange(NE)[None, :] >= ent[:, None].astype(np.int64)  # (Na, Ne)
    wm = wf * (~col_mask.reshape(-1))[:, None].astype(np.float32)

    d_sim, d_ind, _ = _run_device(vis, wm)
    margin_loss = _host_tail(vis, ent, col_mask, d_sim, d_ind)

    ind_dtype = np.int64 if ent.dtype == np.int64 else np.int32
    return d_ind.astype(ind_dtype), d_sim, margin_loss


# revision 21
# speedup vs baseline: 1.0891x; 1.0891x over previous
"""Trainium2 Bass kernel for the DVSA loss function.

Contract: kernel(**inputs) takes the FULL unsharded inputs
(vis_feats (51200,512) f32, word_feats (192,512) f32, entities_length (16,)
int, Nb scalar) and returns the full outputs (D_ind, D_sim, margin_loss),
matching reference semantics.

Strategy: shard vis_feats rows 8 ways (data-parallel over actions);
word_feats is masked (padding entity slots zeroed from entities_length) and
replicated. Each core computes S^T = Wm @ V^T via the tensor engine with the
box-group max/argmax fused on-chip; the tiny loss tail runs on host from the
(Na*Ns, Na*Ne) max/argmax arrays.
"""

import numpy as np

EPS = 1e-5
DELTA = 0.2
VIS_LAM = 1.0

# Geometry (hardcoded per problem spec).
NA, NS, NB, NE, D = 16, 32, 100, 12, 512
N_CORES = 8
ROWS = NA * NS * NB            # 51200
R_CORE = ROWS // N_CORES       # 6400 rows/core
F_CORE = R_CORE // NB          # 64 frames/core
COLS = NA * NE                 # 192
KD = D // 128                  # 4 contraction chunks
CF = 5                         # frames per PSUM chunk (500 rows <= 512 fp32 bank)

# Matmul precision mode: 'f32r' (1 cyc/row), 'f16x2' (hi/lo, 3 cyc/row),
# 'f32' (4 cyc/row, exact).
MODE = "f32r"

_NC_CACHE = {}


def _build_nc(mode=MODE, r_core=R_CORE, nb=NB, cols=COLS, d=D, cf=CF,
              ablate=None, reps=1):
    import concourse.bacc as bacc
    import concourse.bass as bass
    import concourse.tile as tile
    from concourse import mybir

    f32 = mybir.dt.float32
    kd = d // 128
    n_frames = r_core // nb
    ng = (cols + 127) // 128          # column groups, zero-padded to 128 each
    cpad = ng * 128

    nc = bacc.Bacc("TRN2", target_bir_lowering=False, debug=False)

    if mode == "f16x2":
        mdt = mybir.dt.float16
        v_drams = [
            nc.dram_tensor("vth", [d, r_core], mdt, kind="ExternalInput"),
            nc.dram_tensor("vtl", [d, r_core], mdt, kind="ExternalInput"),
        ]
        w_drams = [
            nc.dram_tensor("wth", [kd, 128, cpad], mdt, kind="ExternalInput"),
            nc.dram_tensor("wtl", [kd, 128, cpad], mdt, kind="ExternalInput"),
        ]
    else:
        mdt = f32 if mode == "f32" else mybir.dt.float32r
        v_drams = [nc.dram_tensor("vt", [d, r_core], mdt, kind="ExternalInput")]
        w_drams = [nc.dram_tensor("wt", [kd, 128, cpad], mdt, kind="ExternalInput")]

    u32 = mybir.dt.uint32
    iota_d = nc.dram_tensor("iota", [128, cf * nb + 1], u32, kind="ExternalInput")
    sim_d = nc.dram_tensor("sim0", [128, ng, n_frames], f32, kind="ExternalOutput")

    with tile.TileContext(nc) as tc:
        with (
            tc.tile_pool(name="const", bufs=1) as const_pool,
            tc.tile_pool(name="vin", bufs=6) as vin_pool,
            tc.tile_pool(name="work", bufs=3) as work_pool,
            tc.tile_pool(name="psp", bufs=4, space=bass.MemorySpace.PSUM) as ps_pool,
        ):
            w_sbs = []
            for wi, wd in enumerate(w_drams):
                w_sb = const_pool.tile([128, kd, cpad], mdt, tag=f"w{wi}", name=f"w{wi}")
                for k in range(kd):
                    nc.sync.dma_start(out=w_sb[:, k, :], in_=wd[k])
                w_sbs.append(w_sb)

            iota_sb = const_pool.tile([128, cf * nb + 1], u32)
            nc.sync.dma_start(out=iota_sb[:], in_=iota_d[:])

            osim = const_pool.tile([128, ng, n_frames], f32, name="osim")
            if ablate:
                nc.vector.memset(osim[:], 0.0)

            n_chunks = (n_frames + cf - 1) // cf
            for rep in range(reps):
              for ci0 in range(n_chunks):
                ci = f"{rep}_{ci0}" if reps > 1 else ci0
                f0 = ci0 * cf
                nf = min(cf, n_frames - f0)
                r0 = f0 * nb
                rn = nf * nb

                v_sbs = []
                for vi, vd in enumerate(v_drams):
                    v_sb = vin_pool.tile([128, kd, cf * nb], mdt, tag=f"v{vi}", name=f"v{vi}_{ci}")
                    for k in range(kd):
                        nc.sync.dma_start(out=v_sb[:, k, :rn],
                                          in_=vd[k * 128:(k + 1) * 128, r0:r0 + rn])
                    v_sbs.append(v_sb)

                if mode == "f16x2":
                    # S = Vh@Wh + Vl@Wh + Vh@Wl (drop Vl@Wl, ~2^-22 relative)
                    mm_pairs = [(w_sbs[0], v_sbs[0]), (w_sbs[0], v_sbs[1]),
                                (w_sbs[1], v_sbs[0])]
                else:
                    mm_pairs = [(w_sbs[0], v_sbs[0])]

                if ablate == "dma":
                    continue
                # one PSUM tile spanning ng banks; col-group g -> bank g
                ps = ps_pool.tile([128, ng, 512], f32, tag="ps", name=f"ps_{ci}")
                n_mm = kd * len(mm_pairs)
                for g in range(ng):
                    i_mm = 0
                    for k in range(kd):
                        for (w_sb, v_sb) in mm_pairs:
                            nc.tensor.matmul(
                                ps[:, g, :rn],
                                w_sb[:, k, g * 128:(g + 1) * 128],
                                v_sb[:, k, :rn],
                                start=(i_mm == 0),
                                stop=(i_mm == n_mm - 1),
                            )
                            i_mm += 1

                if ablate == "pe":
                    continue
                # Bit-embed the descending box index into the 7 low mantissa
                # bits: emb = (S & ~0x7F) | (nb-1-box). fp32 max over emb
                # then yields max (top bits, ~2^-17 truncation) AND
                # first-occurrence argmax (low bits) in one reduce.
                emb = work_pool.tile([128, ng, cf * nb], u32, tag="emb", name=f"emb_{ci}")
                embv = emb[:, :, :rn]
                psv = ps[:, :, :rn]
                iota_bcast = bass.AP(
                    tensor=iota_sb.tensor, offset=iota_sb.offset,
                    ap=[[iota_sb.ap[0][0], 128], [0, ng], [1, rn]],
                )
                nc.vector.scalar_tensor_tensor(
                    out=embv, in0=psv.bitcast(u32),
                    scalar=iota_sb[:, cf * nb:cf * nb + 1],
                    in1=iota_bcast,
                    op0=mybir.AluOpType.bitwise_and,
                    op1=mybir.AluOpType.bitwise_or,
                )
                nc.vector.tensor_reduce(
                    osim[:, :, f0:f0 + nf],
                    emb[:, :, :rn].rearrange("p t (f b) -> p t f b", b=nb).bitcast(f32),
                    axis=mybir.AxisListType.X, op=mybir.AluOpType.max,
                )

            nc.sync.dma_start(out=sim_d[:], in_=osim[:])

    nc.compile()
    return nc


def _get_nc(mode=MODE):
    if mode not in _NC_CACHE:
        _NC_CACHE[mode] = _build_nc(mode)
    return _NC_CACHE[mode]


def _split_f16(x):
    hi = x.astype(np.float16)
    lo = (x - hi.astype(np.float32)).astype(np.float16)
    return hi, lo


def _prep_in_maps(vis, wm, mode=MODE):
    """vis: (ROWS, D) f32; wm: masked word_feats (COLS, D) f32."""
    iota_desc = np.empty((128, CF * NB + 1), dtype=np.uint32)
    iota_desc[:, :CF * NB] = np.tile(
        np.arange(NB - 1, -1, -1, dtype=np.uint32), CF)[None, :]
    iota_desc[:, CF * NB] = np.uint32(0xFFFFFF80)
    ng = (COLS + 127) // 128
    wmp = np.zeros((ng * 128, D), dtype=np.float32)
    wmp[:COLS] = wm
    wmt = np.ascontiguousarray(wmp.T).reshape(KD, 128, ng * 128)
    in_maps = []
    for c in range(N_CORES):
        vt = np.ascontiguousarray(vis[c * R_CORE:(c + 1) * R_CORE].T)
        m = {"iota": iota_desc}
        if mode == "f16x2":
            vh, vl = _split_f16(vt)
            wh, wl = _split_f16(wmt)
            m.update({"vth": vh, "vtl": vl, "wth": wh, "wtl": wl})
        else:
            m.update({"vt": vt, "wt": wmt})
        in_maps.append(m)
    return in_maps


def _run_device(vis, wm, mode=MODE, trace=False):
    from concourse.bass_utils import run_bass_kernel_spmd

    nc = _get_nc(mode)
    in_maps = _prep_in_maps(vis, wm, mode)
    res = run_bass_kernel_spmd(nc, in_maps, list(range(N_CORES)), trace=trace)
    embs = []
    for c in range(N_CORES):
        st = res.results[c]["sim0"]                           # [128, ng, 64]
        full = np.concatenate([st[:, g, :] for g in range(st.shape[1])], axis=0)
        embs.append(full[:COLS].T)                            # [64, 192]
    raw = np.ascontiguousarray(np.concatenate(embs, axis=0)).view(np.uint32)
    d_sim = (raw & np.uint32(0xFFFFFF80)).view(np.float32)    # (512, 192)
    d_ind = (NB - 1) - (raw & np.uint32(0x7F)).astype(np.int64)
    return d_sim, d_ind, res


def _host_tail(vis, ent, col_mask, d_sim, d_ind):
    """Mirror of the reference loss tail, in float64, from D_sim/D_ind."""
    na, ns, ne = NA, NS, NE
    ar = np.arange(na)

    div_vec = np.where(ent == 0, 1, ent).astype(np.float64)

    # --- vis loss ---
    ds4 = d_sim.reshape(na, ns, na, ne)
    di4 = d_ind.reshape(na, ns, na, ne)
    sim_scr = ds4[ar, :, ar, :].astype(np.float64)            # (Na, Ns, Ne)
    maxind = di4[ar, :, ar, :]                                # (Na, Ns, Ne)
    smin = sim_scr.min(axis=1, keepdims=True)
    smax = sim_scr.max(axis=1, keepdims=True)
    sim_n = (sim_scr - smin) / (smax - smin + EPS)

    vcls = vis[maxind.reshape(-1)].reshape(na, ns, ne, D).astype(np.float64)
    vcls = vcls / (np.linalg.norm(vcls, axis=3, keepdims=True) + EPS)
    vcls = vcls * sim_n[..., None]
    v1 = vcls.transpose(0, 2, 1, 3)                           # (Na, Ne, Ns, D)
    vis_mat = 1.0 - np.einsum("aesd,aetd->aest", v1, v1)
    eye = np.eye(ns, dtype=bool)
    mvis = col_mask[:, :, None, None] | eye[None, None, :, :]
    vis_mat = np.where(mvis, 0.0, vis_mat)
    dem = np.count_nonzero(vis_mat)
    vis_loss = vis_mat.sum() / dem

    # --- margin loss ---
    s = d_sim.reshape(na, ns, na * ne).astype(np.float64)
    smin2 = s.min(axis=1, keepdims=True)
    smax2 = s.max(axis=1, keepdims=True)
    s = s * ((s - smin2) / (smax2 - smin2 + EPS))
    sf = s.reshape(na, ns, na, ne).sum(-1) / div_vec          # (Na, Ns, Na)
    sf_diag = sf[ar, :, ar]                                   # (Na, Ns)
    term1 = np.maximum(sf - sf_diag.T[None, :, :] + DELTA, 0.0).mean(0).T
    term2 = np.maximum(sf - sf_diag[:, :, None] + DELTA, 0.0).mean(2)
    frame_score = term1 + term2
    margin_loss = (frame_score.mean() + VIS_LAM * vis_loss) * 10.0
    return np.float32(margin_loss)


def kernel(vis_feats, word_feats, entities_length, Nb):
    vis = np.ascontiguousarray(np.asarray(vis_feats, dtype=np.float32))
    wf = np.asarray(word_feats, dtype=np.float32)
    ent = np.asarray(entities_length)
    assert int(Nb) == NB and vis.shape == (ROWS, D) and wf.shape == (COLS, D)

    col_mask = np.arange(NE)[None, :] >= ent[:, None].astype(np.int64)  # (Na, Ne)
    wm = wf * (~col_mask.reshape(-1))[:, None].astype(np.float32)

    d_sim, d_ind, _ = _run_device(vis, wm)
    margin_loss = _host_tail(vis, ent, col_mask, d_sim, d_ind)

    ind_dtype = np.int64 if ent.dtype == np.int64 else np.int32
    return d_ind.astype(ind_dtype), d_sim, margin_loss
